# revision 1
# baseline (speedup 1.0000x reference)
"""Multi-head self-attention kernel for Trainium2 (8 NeuronCores).

Problem: q,k,v [4000, 4096] fp32; the module attends q against itself
(k and v are ignored by the reference). 32 heads of dim 128.

Sharding: tensor-parallel over heads - each of the 8 cores owns 4 heads
(a [4000, 512] column slice of q) and computes its full attention
independently; the host concatenates the per-core outputs (the
"all-gather" along the feature axis).

v2 algorithm (per head, all fp16 compute):
  The score matrix S = q q^T / sqrt(hd) is SYMMETRIC, so P = exp(S*scale - 8.5)
  is symmetric too. Only the upper-triangle 128x128 blocks are computed:
    - mm1 row-strips: S_T[c, c..31] on the PE (fp16, 1 cycle/row at any width)
    - exp on ACT into fp16 strips (halves the ACT work vs the full matrix)
    - the lower-triangle tiles are produced by DMA-engine blockwise
      transposes of the fp16 strips (InstDmaTransposeAnt, idle device)
  mm2 runs in natural orientation: O[q in unit u, :] = sum_r tile(r,u)^T @ vr[r]
  with vr[r] = [v block r | ones column]. The ones column makes the PSUM
  accumulator's last column the softmax denominator l[q] exactly (the -8.5
  shift cancels in the ratio), so there is no separate row-sum pass and no
  output transpose epilogue: out = po[:, :128] * (1/po[:, 128]).

Scheduling: one software pipeline over all 32 (head, stage) pairs with
output bands lagging score stages by 2 iterations, crossing head
boundaries (the previous head's last two bands fill the PE while the next
head's prologue exps run; subtile WAR deps make the strip reuse safe).
Each band's mirrors are issued one iteration ahead. tile_wait_until
pacing (pace=1.4, startup=12us) gives the Tile scheduler - whose legacy
cost model treats DMA transfer time as free latency - per-iteration
minimum start times, preventing it from clumping work in ways that
serialize on the (actually exclusive) DMA engines; this also keeps the
PE p-state warm; the final head's iterations are paced at 0.8x
(no prefetch DMA there, real deps take over). 498.7us vs the 539.4us
fp32r/full-matrix baseline.
"""

import numpy as np

N = 4000
D_MODEL = 4096
NUM_HEADS = 32
HD = 128
N_CORES = 8
H_PER_CORE = NUM_HEADS // N_CORES          # 4
D_CORE = H_PER_CORE * HD                   # 512
P = 128
U = 32                                     # 32 row/col units of 128 (4096 padded)
NPAD = U * P                               # 4096
SCALE = 1.0 / np.sqrt(np.float32(HD))
EXP_SHIFT = 8.5

_CACHE = {}
EMIT_LOG = []  # (engine, opcode-ish, label) in emission order, for debugging

import os

# Scheduling knobs (tuned via TimelineSim sweeps; envs are optional overrides)
OPT = {
    "pace": float(os.environ.get("K_PACE", "1.34")),  # 0 disables pacing
    "mir_split": os.environ.get("K_MIRSPLIT", "none"),  # head|all|none
    "startup": float(os.environ.get("K_STARTUP", "12.0")),
}


def _build():
    import concourse.bacc as bacc
    import concourse.tile as tile
    from concourse import mybir

    f32 = mybir.dt.float32
    fp16 = mybir.dt.float16
    Exp = mybir.ActivationFunctionType.Exp

    nc = bacc.Bacc("TRN2", target_bir_lowering=False, debug=False)
    q_in = nc.declare_dram_parameter("q", [N, D_CORE], f32, isOutput=False)
    o_out = nc.declare_dram_parameter("out", [N, D_CORE], f32, isOutput=True)

    with tile.TileContext(nc) as tc:
        with (
            tc.tile_pool(name="singles", bufs=1) as singles,
            tc.tile_pool(name="qn16", bufs=1) as qn_pool,
            tc.tile_pool(name="qT", bufs=2) as qT_pool,
            tc.tile_pool(name="vr", bufs=int(os.environ.get("K_VR", "1"))) as vr_pool,
            tc.tile_pool(name="strip", bufs=1) as strip_pool,
            tc.tile_pool(name="mir", bufs=1) as mir_pool,
            tc.tile_pool(name="ob", bufs=int(os.environ.get("K_OB", "2"))) as ob_pool,
            tc.tile_pool(name="rrec", bufs=4) as r_pool,
            tc.tile_pool(name="ps_s", bufs=2, space="PSUM") as ps_s_pool,
            tc.tile_pool(name="ps_o", bufs=int(os.environ.get("K_PSO", "2")), space="PSUM") as ps_o_pool,
        ):
            exp_bias = singles.tile([P, 1], f32)
            nc.vector.memset(exp_bias, -float(EXP_SHIFT))

            # per-partition masks for the vr ones-column (avoid partition-
            # sliced strided memsets, which the BIR verifier rejects)
            i32 = mybir.dt.int32
            pidx = singles.tile([P, 1], i32)
            nc.gpsimd.iota(pidx, [[0, 1]], base=0, channel_multiplier=1)
            ones_col = singles.tile([P, 1], fp16)
            mask31 = singles.tile([P, 1], fp16)
            Lt = mybir.AluOpType.is_lt
            nc.vector.tensor_scalar(ones_col, pidx, 99999, None, op0=Lt)
            nc.vector.tensor_scalar(mask31, pidx, N - (U - 1) * P, None, op0=Lt)

            def alloc_input():
                qn = qn_pool.tile([P, NPAD], fp16, tag="qn")
                qT = qT_pool.tile([P, NPAD], fp16, tag="qT")
                return qn, qT

            def emit_chunk(h, ch, qn, qT):
                """Load one 8-block chunk of q[:, head cols] f32 -> qn fp16
                (cast via gpsimd SWDGE), then its blockwise qT transpose."""
                hs = slice(h * HD, (h + 1) * HD)
                b0 = ch * 8
                b1 = U - 1 if ch == 3 else b0 + 8  # last block separate
                with dma_paced(1.46):
                    nc.gpsimd.dma_start(
                        out=qn[:, b0 * P : b1 * P].rearrange(
                            "p (c d) -> p c d", c=b1 - b0
                        ),
                        in_=q_in[b0 * P : b1 * P, hs].rearrange(
                            "(c p) d -> p c d", p=P
                        ),
                    )
                if ch == 3:
                    nc.vector.memset(qn[:, (U - 1) * P :], 0.0)
                    nc.gpsimd.dma_start(
                        out=qn[: N - (U - 1) * P, (U - 1) * P :],
                        in_=q_in[(U - 1) * P : N, hs],
                    )
                cs = slice(b0 * P, (b0 + 8) * P)
                qeng = (
                    nc.scalar if os.environ.get("K_QTQ", "sp") == "act" else nc.sync
                )
                with dma_paced(0.9 + 0.4):
                    qeng.dma_start(
                        out=qT[:, cs].rearrange("p (c d) -> p c d", c=8),
                        in_=qn[:, cs],
                        transpose=True,
                    )
                EMIT_LOG.append(("SPT", "qT", f"h{h}c{ch}"))

            def emit_vr(h, qn):
                """vr[:, r, 0:128] = q rows of block r (fp16), col 128 = 1.0
                (0 in the padding rows of the last block so padded keys add
                nothing to the softmax denominator)."""
                vr = vr_pool.tile([P, U, HD + 1], fp16, tag="vr")
                if h < 2:  # each buffer: init ones column once
                    for r in range(U - 1):
                        nc.vector.tensor_copy(vr[:, r, HD : HD + 1], ones_col)
                    nc.vector.tensor_copy(vr[:, U - 1, HD : HD + 1], mask31)
                for r in range(U):
                    nc.vector.tensor_copy(
                        vr[:, r, 0:HD], qn[:, r * P : (r + 1) * P]
                    )
                return vr

            strips = {}  # (head, row) -> strip tile
            mirs = {}  # (head, unit) -> mirror tile

            def emit_mirror(h, u, piece=None):
                """mirror transpose of strip u minus its diagonal block:
                the lower-triangle tiles [k-part, u-cols] for k > u.
                Issued one iteration before its band consumes it, when the
                strip's exp is already complete (SP never head-blocks).
                piece="A"/"B" splits at strip offset 3072 so each head's
                first band can get most of its mirrors before the previous
                head's last band (which write-protects the strips' tails)
                completes."""
                j = u % 4
                W = (U - u) * P
                if piece != "B":
                    mir = mir_pool.tile(
                        [P, (U - 1 - j) * P], fp16, tag=f"m{j}"
                    )
                    mirs[(h, u)] = mir
                else:
                    mir = mirs[(h, u)]
                lo = P if piece != "B" else 3072
                hi = W if piece != "A" else min(3072, W)
                if hi <= lo:
                    return
                meng = (
                    nc.scalar
                    if os.environ.get("K_MIRQ", "sp") == "alt" and u % 2
                    else nc.sync
                )
                with dma_paced((hi - lo) // P * 0.112 + 0.4):
                    meng.dma_start(
                        out=mir[:, lo - P : hi - P].rearrange(
                            "p (c d) -> p c d", c=(hi - lo) // P
                        ),
                        in_=strips[(h, u)][:, lo:hi],
                        transpose=True,
                    )
                EMIT_LOG.append(("SPT", "mir", f"h{h}u{u}{piece or ''}"))

            def emit_row(h, qT, c):
                """mm1 + exp for upper-triangle row c (skipping the 96 dead
                padded q-columns at the end; the strip tail is zeroed once on
                head 0 and afterwards holds stale-but-finite exp values that
                only ever reach padded k-partitions x zero vr rows)."""
                W = (U - c) * P
                WV = W - (NPAD - N)
                st = strip_pool.tile([P, W], fp16, tag=f"strip{c}")
                strips[(h, c)] = st
                if h == 0:
                    nc.vector.memset(st[:, WV:W], 0.0)
                off = 0
                CW = int(os.environ.get("K_CHUNK", "1536"))
                while off < WV:
                    w = min(CW, WV - off)
                    ps = ps_s_pool.tile([P, CW], f32, tag="ps_s")
                    o2 = 0
                    while o2 < w:
                        ww = min(512, w - o2)
                        nc.tensor.matmul(
                            ps[:, o2 : o2 + ww],
                            lhsT=qT[:, c * P : (c + 1) * P],
                            rhs=qT[:, c * P + off + o2 : c * P + off + o2 + ww],
                            start=True,
                            stop=True,
                        )
                        o2 += ww
                    nc.scalar.activation(
                        st[:, off : off + w],
                        ps[:, :w],
                        Exp,
                        scale=float(SCALE),
                        bias=exp_bias[:, :],
                    )
                    EMIT_LOG.append(("ACT", "exp", f"h{h}r{c}o{off}"))
                    off += w

            def emit_unit(h, vr, u):
                """32-step mm2 accumulation + epilogue + per-unit out DMA."""
                hs = slice(h * HD, (h + 1) * HD)
                mir = mirs.get((h, u))
                po = ps_o_pool.tile([P, 512], f32, tag="ps_o")
                for r in range(U):
                    if r <= u:
                        lhsT = strips[(h, r)][:, (u - r) * P : (u - r + 1) * P]
                    else:
                        lhsT = mir[:, (r - u - 1) * P : (r - u) * P]
                    nc.tensor.matmul(
                        po[:, 0 : HD + 1],
                        lhsT=lhsT,
                        rhs=vr[:, r, :],
                        start=(r == 0),
                        stop=(r == U - 1),
                    )
                r_t = r_pool.tile([P, 1], f32, tag=f"r{u % 4}")
                EMIT_LOG.append(("DVEr", "recip", f"h{h}u{u}"))
                nc.vector.reciprocal(r_t, po[:, HD : HD + 1])
                j = u % 4
                if j == 0:
                    emit_unit.ob = ob_pool.tile([P, 4 * P], f32, tag="ob")
                ob = emit_unit.ob
                nc.vector.tensor_scalar_mul(
                    ob[:, j * P : (j + 1) * P], po[:, 0:HD], r_t[:, 0:1]
                )
                if j == 3:
                    g = u // 4
                    with dma_paced(0.73 + 0.3):
                        if g < 7:
                            nc.gpsimd.dma_start(
                                out=o_out[
                                    g * 512 : (g + 1) * 512, hs
                                ].rearrange("(c p) d -> p c d", p=P),
                                in_=ob.rearrange("p (c d) -> p c d", c=4),
                            )
                        else:
                            nc.gpsimd.dma_start(
                                out=o_out[
                                    7 * 512 : 7 * 512 + 3 * P, hs
                                ].rearrange("(c p) d -> p c d", p=P),
                                in_=ob[:, 0 : 3 * P].rearrange(
                                    "p (c d) -> p c d", c=3
                                ),
                            )
                            nc.gpsimd.dma_start(
                                out=o_out[31 * P : N, hs],
                                in_=ob[: N - 31 * P, 3 * P : 4 * P],
                            )

            # One continuous pipeline over all 32 global stages/bands with
            # bands lagging stages by 2, crossing head boundaries: head h's
            # last two bands are the PE filler under head h+1's prologue
            # exps. Mirrors for band b+1 are issued at iteration b (strips'
            # exps done -> SP never head-blocks); each head's FIRST band
            # gets split mirrors: piece A (blocks < strip offset 3072) a
            # full iteration early, piece B (the strip tails, which carry a
            # WAR against the previous head's last band) at its own
            # iteration start.
            def emit_band_mirrors(b, piece=None):
                if not 0 <= b < 32:
                    return
                h_m, g_m = divmod(b, 8)
                for j in range(4):
                    u = 4 * g_m + j
                    if u < U - 1:
                        emit_mirror(h_m, u, piece)

            # Iteration pacing (tile_wait_until): minimum start times keep
            # the scheduler - whose legacy cost model treats DMA transfer
            # time as free latency - from hoisting work into clumps that
            # serialize on the (actually exclusive) DMA engines. Times in us.
            import contextlib

            def mm1_us(s):
                if s >= 32:
                    return 0.0
                g = s % 8
                return sum(U - c for c in range(4 * g, 4 * g + 4)) * 128 * 0.4167e-3

            def act_us(s):
                if s >= 32:
                    return 0.0
                g = s % 8
                units = sum(U - c for c in range(4 * g, 4 * g + 4))
                return units * 128 * 0.8333e-3 + 3 * 0.19

            MIRLAT = float(os.environ.get("K_MIRLAT", "0.53"))

            def dma_us(nb):
                if nb >= 32:
                    return 0.0
                g = nb % 8
                mir = sum(
                    (U - 1 - (4 * g + j)) * 0.112 + MIRLAT
                    for j in range(4)
                    if 4 * g + j < U - 1
                )
                pre0 = int(os.environ.get("K_PRE0", "0"))
                extra = (1.46 + 0.9) if pre0 <= g <= pre0 + 3 else 0.0
                return mir + extra + 0.73  # + band out

            T = OPT["startup"]  # startup: input chunks + prologue
            PACE = OPT["pace"] > 0
            dclk = [0.0]  # DMA-device clock (us): serializes DMA holds
            DPACE = os.environ.get("K_DPACE", "0") == "1"

            def dma_paced(hold_us):
                """pace a DMA instruction at the DMA-device clock"""
                if not (PACE and DPACE):
                    return contextlib.nullcontext()
                t = dclk[0]
                dclk[0] = t + hold_us
                return tc.tile_wait_until(t * 1e-3)

            def paced(t_us):
                if not PACE:
                    return contextlib.nullcontext()
                return tc.tile_wait_until(t_us * 1e-3)  # arg in ms

            LAG = int(os.environ.get("K_LAG", "2"))
            inputs = {}
            qn, qT = alloc_input()
            for ch in range(4):
                emit_chunk(0, ch, qn, qT)
            inputs[0] = (qT, emit_vr(0, qn))
            for s in range(LAG):
                for c in range(4 * s, 4 * s + 4):
                    emit_row(0, qT, c)
            emit_band_mirrors(0)
            for b in range(32):
                h_b, g_b = divmod(b, 8)
                s = b + LAG
                dclk[0] = max(dclk[0], T)
                with paced(T):
                    if (g_b == 0 and OPT["mir_split"] != "none") or OPT[
                        "mir_split"
                    ] == "all":
                        emit_band_mirrors(b, "B")  # strip tails now WAR-free
                    qT_b, vr_b = inputs[h_b]
                    for j in range(4):
                        emit_unit(h_b, vr_b, 4 * g_b + j)
                        if s < 32:
                            h_s, g_s = divmod(s, 8)
                            emit_row(h_s, inputs[h_s][0], 4 * g_s + j)
                    nb = b + 1
                    if OPT["mir_split"] == "all":
                        pc = "A"
                    elif OPT["mir_split"] == "head" and nb % 8 == 0:
                        pc = "A"
                    else:
                        pc = None
                    emit_band_mirrors(nb, pc)
                    # prefetch next head's input during early bands
                    if h_b + 1 < H_PER_CORE:
                        if g_b == 0:
                            nqn, nqT = alloc_input()
                            inputs[h_b + 1] = (nqT, None)
                            qn = nqn
                        if g_b <= 3:
                            emit_chunk(h_b + 1, g_b, qn, inputs[h_b + 1][0])
                        elif g_b == 7:
                            # vr single-buffered: rebuild after band (h,7)
                            inputs[h_b + 1] = (
                                inputs[h_b + 1][0],
                                emit_vr(h_b + 1, qn),
                            )
                actcap = float(os.environ.get("K_ACTCAP", "99"))
                bandc = float(os.environ.get("K_BANDC", "6.9"))
                endf = (
                    float(os.environ.get("K_ENDF", "0.8"))
                    if b >= int(os.environ.get("K_ENDB", "22"))
                    else 1.0
                )
                if b >= int(os.environ.get("K_ENDB2", "32")):
                    endf = float(os.environ.get("K_ENDF2", "0.8"))
                if g_b < int(os.environ.get("K_EARLYG", "4")):
                    endf *= float(os.environ.get("K_EARLYF", "1.14"))
                if g_b >= int(os.environ.get("K_LATEG", "8")):
                    endf = min(endf, float(os.environ.get("K_LATEF", "1.0")))
                T += endf * OPT["pace"] * max(
                    bandc + mm1_us(s), min(act_us(s), actcap), dma_us(b + 1)
                ) + float(os.environ.get("K_SLACK", "0"))

    nc.compile()
    return nc


def _get_nc():
    if "nc" not in _CACHE:
        _CACHE["nc"] = _build()
    return _CACHE["nc"]


def _get_runner():
    """Build (once) a jitted 8-core SPMD executor for the compiled program.

    Mirrors concourse.bass2jax.run_bass_via_pjrt but caches the jitted
    callable so repeat kernel() calls skip retracing/recompilation.
    """
    if "runner" in _CACHE:
        return _CACHE["runner"]

    import jax
    import numpy as _np
    from jax.sharding import Mesh, PartitionSpec
    from jax.experimental.shard_map import shard_map
    from concourse import mybir
    from concourse import bass2jax

    nc = _get_nc()
    bass2jax.install_neuronx_cc_hook()

    in_names, out_names, out_avals, zero_outs = [], [], [], []
    for alloc in nc.m.functions[0].allocations:
        if not isinstance(alloc, mybir.MemoryLocationSet):
            continue
        name = alloc.memorylocations[0].name
        pname = nc.partition_id_tensor.name if nc.partition_id_tensor else None
        if alloc.kind == "ExternalInput":
            if name != pname:
                in_names.append(name)
        elif alloc.kind == "ExternalOutput":
            shape = tuple(alloc.tensor_shape)
            dtype = mybir.dt.np(alloc.dtype)
            out_names.append(name)
            out_avals.append(jax.core.ShapedArray(shape, dtype))
            zero_outs.append((shape, dtype))
    n_params = len(in_names)
    n_outs = len(out_avals)
    all_names = in_names + out_names
    pname = nc.partition_id_tensor.name if nc.partition_id_tensor else None
    if pname is not None:
        all_names = all_names + [pname]

    def _body(*args):
        operands = list(args)
        if pname is not None:
            operands.append(bass2jax.partition_id_tensor())
        outs = bass2jax._bass_exec_p.bind(
            *operands,
            out_avals=tuple(out_avals),
            in_names=tuple(all_names),
            out_names=tuple(out_names),
            lowering_input_output_aliases=(),
            sim_require_finite=True,
            sim_require_nnan=True,
            nc=nc,
        )
        return tuple(outs)

    devices = jax.devices()[:N_CORES]
    mesh = Mesh(_np.asarray(devices), ("core",))
    specs = (PartitionSpec("core"),) * (n_params + n_outs)
    sharded = jax.jit(
        shard_map(
            _body,
            mesh=mesh,
            in_specs=specs,
            out_specs=(PartitionSpec("core"),) * n_outs,
            check_rep=False,
        ),
        donate_argnums=tuple(range(n_params, n_params + n_outs)),
        keep_unused=True,
    )

    def run(per_core_inputs):
        concat_in = [
            _np.concatenate([m[nm] for m in per_core_inputs], axis=0)
            for nm in in_names
        ]
        concat_zero = [
            _np.zeros((N_CORES * s[0], *s[1:]), dt) for s, dt in zero_outs
        ]
        out_arrs = sharded(*concat_in, *concat_zero)
        return [
            {
                nm: _np.asarray(out_arrs[i]).reshape(
                    N_CORES, *out_avals[i].shape
                )[c]
                for i, nm in enumerate(out_names)
            }
            for c in range(N_CORES)
        ]

    _CACHE["runner"] = run
    return run


def kernel(**inputs: np.ndarray) -> np.ndarray:
    q = np.ascontiguousarray(np.asarray(inputs["q"], dtype=np.float32))
    assert q.shape == (N, D_MODEL)

    in_maps = [
        {"q": np.ascontiguousarray(q[:, c * D_CORE : (c + 1) * D_CORE])}
        for c in range(N_CORES)
    ]
    try:
        run = _get_runner()
        results = run(in_maps)
    except Exception:
        # fall back to the stock SPMD runner (pays a re-jit per call)
        from concourse.bass_utils import run_bass_kernel_spmd

        _CACHE.pop("runner", None)
        res = run_bass_kernel_spmd(_get_nc(), in_maps, list(range(N_CORES)))
        results = res.results
    out = np.concatenate([results[c]["out"] for c in range(N_CORES)], axis=1)
    return out.astype(np.float32)

